# revision 1
# baseline (speedup 1.0000x reference)
"""CrossModalAdaptiveFusion Trainium2 kernel (8 NeuronCores, SPMD).

Sharding: the 32^3 volume is split into 8 H-slabs of 4 planes (+1-plane halo,
host-padded), so the depthwise conv, GroupNorm reduction and the final 1x1x1
projection all stay core-local. Cross-core traffic is four tiny collectives:
AllReduce of per-channel sums (global avg-pool), AllGather of the kernel-MLP
hidden (h1) and of the 768x27 dynamic kernels, and AllReduce of 12x2
GroupNorm stats.

Per core: the text/attention path is host-folded into a single 768x768 matrix
(softmax over one key is exactly 1, so attn == v), the kernel-MLP is
row-sharded (each core streams 1/8 of the 63M-param kn_w2), the depthwise
3x3x3 conv is split between the PE (diagonal-matmul accumulation in PSUM) and
the DVE (scalar_tensor_tensor FMA chain), GroupNorm is folded into a
per-channel affine applied to the conv output, and the 1x1x1 conv is a
768x768 x 4096-voxel GEMM. float32r (full-rate reduced-precision fp32
matmul) is used for the heavy matmuls; everything else is fp32.
"""
import sys

sys.path.insert(0, "/opt/trn_rl_repo")

import numpy as np

import concourse.bass as bass
import concourse.mybir as mybir
from concourse import tile
from concourse import bass_utils

F32 = mybir.dt.float32
F32R = mybir.dt.float32r
BF16 = mybir.dt.bfloat16
AO = mybir.AluOpType
ACTF = mybir.ActivationFunctionType

C = 768
G = 12
GD = C // G          # 64 channels per group
H = W = D = 32
NCORES = 8
HS = H // NCORES     # 4 H-planes per core
NB = C // 128        # 6 channel blocks
PH, PW, PD = HS + 2, W + 2, D + 2   # padded slab dims: 6 x 34 x 34
SLABF = PH * PW * PD                # 6936 free elements per channel
NVOX = HS * W * D                   # 4096 voxels per core
NG_TOT = GD * H * W * D             # element count per GroupNorm group
H1_SH = 4 * C // NCORES             # 384 hidden rows per core
KP_SH = C * 27 // NCORES            # 2592 kernel params per core
EPS = 1e-5

# Precision of the conv + final GEMM path: bf16 runs the PE at full rate
# with no per-matmul weight-reload penalty and halves the slab/xc footprint;
# fp32r keeps fp32 storage with a reduced-precision multiply.
CONV_BF16 = True
W2_BF16 = True

# Tap split between engines: DVE runs an FMA chain, the PE runs diagonal
# matmuls accumulating in PSUM.
DVE_TAPS = list(range(7)) if CONV_BF16 else list(range(10))
PE_TAPS = [t for t in range(27) if t not in DVE_TAPS]
CDT = BF16 if CONV_BF16 else F32R          # conv/GEMM data dtype on device
CDT_IN = BF16 if CONV_BF16 else F32        # dram dtype of slab/convT
W2DT = BF16 if W2_BF16 else F32            # dram/stream dtype of kn_w2

_BUILD_CACHE = {}
_SKIP = set()   # debug: phase names to stub out ("A","W2","PE","DVE","GEMM")
_STOP = ""      # debug: stop emission after phase "A"/"B"/"C"


def split_multi_waits(nc, max_waits=1):
    """The walrus build in this container accepts at most one sync wait per
    instruction; Tile attaches several. Split the extras into standalone
    single-wait EventSemaphore instructions on the same engine."""
    for bb in nc.main_func.blocks:
        new_list = []
        for inst in bb.instructions:
            si = inst.sync_info
            waits = list(si.on_wait) if si and si.on_wait else []
            if len(waits) > max_waits:
                keep, move = waits[:max_waits], waits[max_waits:]
                for k, w in enumerate(move):
                    ev = mybir.InstEventSemaphore(
                        name=f"{inst.name}-ws{k}", ins=[], outs=[])
                    ev.engine = inst.engine
                    ev.sync_info = mybir.SyncInfo(on_wait=[w], on_update=[])
                    new_list.append(ev)
                si.on_wait = keep
            new_list.append(inst)
        bb.instructions[:] = new_list


def _tap_view(slab_r, t):
    """Shifted [128, 4, 32, 32] view of the padded slab for tap t."""
    a, b, c3 = t // 9, (t // 3) % 3, t % 3
    return slab_r[:, a:a + HS, b:b + W, c3:c3 + D]


def build_program(with_collectives=True):
    nc = bass.Bass("TRN2", target_bir_lowering=False, debug=False,
                   num_devices=NCORES)

    def din(name, shape, dt=F32):
        return nc.dram_tensor(name, shape, dt, kind="ExternalInput").ap()

    io = {}
    io["slab_d"] = din("slab", [C, SLABF], CDT_IN)       # padded H-slab
    io["aT_d"] = din("aT", [C, C])               # (Wo@Wv@Wt).T
    io["text_d"] = din("textv", [128, 6])        # text chunks, col k = t[128k+p]
    io["bcomp_d"] = din("bcomp", [128, 6])       # folded attn bias chunks
    io["w1T_d"] = din("w1T", [2 * C, H1_SH])     # kn_w1 row-shard, transposed
    io["b1_d"] = din("b1", [128, 3])             # kn_b1 shard chunks
    io["w2T_d"] = din("w2T", [4 * C, KP_SH], W2DT)     # kn_w2 row-shard, transposed
    io["b2_d"] = din("b2", [1, KP_SH])           # kn_b2 shard
    io["modT_d"] = din("modT", [2 * C, C])       # mod_w.T
    io["modb_d"] = din("modb", [128, 6])         # mod_b chunks
    io["convT_d"] = din("convT", [C, C], CDT_IN)         # conv_w.T  [in, out]
    io["convb_d"] = din("convb", [128, 6])       # conv_b chunks
    io["gnw_d"] = din("gnw", [C])                # gn_w
    io["gnb_d"] = din("gnb", [C])                # gn_b
    io["eye_d"] = din("eye", [128, 128])         # identity, for diag builds
    io["ind_d"] = din("ind", [128, G * NB])      # channel->group indicator
    io["sel_d"] = din("sel", [G, C])             # group->channel selector
    io["out_d"] = nc.dram_tensor("out", [C, NVOX], F32,
                                 kind="ExternalOutput").ap()

    with tile.TileContext(nc) as tc:
        _emit(nc, tc, io, with_collectives)

    split_multi_waits(nc)
    return nc


def _emit(nc, tc, io, with_collectives):
    slab_d = io["slab_d"]
    out_d = io["out_d"]
    RG = [list(range(NCORES))]

    def cc(kind, op, in_ap, out_ap):
        if with_collectives:
            nc.gpsimd.collective_compute(
                kind, op, replica_groups=RG,
                ins=[in_ap.opt()], outs=[out_ap.opt()])
        else:
            shp = in_ap.shape
            nc.gpsimd.dma_start(
                out_ap[tuple(slice(0, s) for s in shp)], in_ap[:])

    small_cm = tc.tile_pool(name="small", bufs=1)
    small = small_cm.__enter__()

    vcpart = small.tile([128, 6], F32, tag="vcpart", name="vcpart")
    combined = small.tile([128, 12], F32, tag="combined", name="combined")
    mod_sb = small.tile([128, 6], F32, tag="mod", name="mod")
    h1full = small.tile([128, 24], BF16 if W2_BF16 else F32R,
                    tag="h1full", name="h1full")
    keff = small.tile([128, 27 * NB], F32, tag="keff", name="keff")
    chsum = small.tile([128, 24], F32, tag="chsum", name="chsum")
    chsq = small.tile([128, 24], F32, tag="chsq", name="chsq")
    eye_sb = small.tile([128, 128], F32, tag="eye", name="eye")
    gnw_sb = small.tile([128, 6], F32, tag="gnw", name="gnw")
    gnb_sb = small.tile([128, 6], F32, tag="gnb", name="gnb")
    convb_sb = small.tile([128, 6], F32, tag="convb", name="convb")
    modb_sb = small.tile([128, 6], F32, tag="modb", name="modb")
    b1_sb = small.tile([128, 3], F32, tag="b1", name="b1")
    b2_sb = small.tile([1, KP_SH], F32, tag="b2", name="b2")
    kp_sb = small.tile([1, KP_SH], F32, tag="kp", name="kp")
    text_sb = small.tile([128, 6], F32, tag="text", name="text")
    bcomp_sb = small.tile([128, 6], F32, tag="bcomp", name="bcomp")
    ind_sb = small.tile([128, G * NB], F32, tag="ind", name="ind")
    sel_sb = small.tile([G, 128 * NB], F32, tag="sel", name="sel")
    s_sb = small.tile([128, 6], F32, tag="s", name="s")
    t_sb = small.tile([128, 6], CDT, tag="t", name="t")
    gv_sb = small.tile([G, 4], F32, tag="gv", name="gv")
    bpp_sb = small.tile([128, 6], F32, tag="bpp", name="bpp")
    h1sh = small.tile([128, 3], F32, tag="h1sh", name="h1sh")
    chstats = small.tile([128, 2], F32, tag="chstats", name="chstats")
    gstat = small.tile([G, 2], F32, tag="gstat_sb", name="gstat_sb")

    nc.sync.dma_start(eye_sb[:], io["eye_d"][:])
    nc.sync.dma_start(gnw_sb[:], io["gnw_d"].rearrange("(k p) -> p k", p=128))
    nc.sync.dma_start(gnb_sb[:], io["gnb_d"].rearrange("(k p) -> p k", p=128))
    nc.sync.dma_start(convb_sb[:], io["convb_d"][:])
    nc.sync.dma_start(modb_sb[:], io["modb_d"][:])
    nc.sync.dma_start(b1_sb[:], io["b1_d"][:])
    nc.sync.dma_start(b2_sb[:], io["b2_d"][:])
    nc.sync.dma_start(text_sb[:], io["text_d"][:])
    nc.sync.dma_start(bcomp_sb[:], io["bcomp_d"][:])
    nc.sync.dma_start(ind_sb[:], io["ind_d"][:])
    nc.sync.dma_start(sel_sb[:], io["sel_d"][:])

    dram_cm = tc.tile_pool(name="dram", bufs=1, space="DRAM")
    dram = dram_cm.__enter__()

    # ---------------- Phase A: per-channel sums for the avg-pool ----------
    with tc.tile_pool(name="slabA", bufs=2) as slabA_pool:
        for b in range(NB if "A" not in _SKIP else 0):
            st = slabA_pool.tile([128, SLABF], CDT_IN, tag="slabA", name="slabA")
            nc.sync.dma_start(st[:], slab_d[128 * b:128 * (b + 1), :])
            sr = st.rearrange("p (h w d) -> p h w d", h=PH, w=PW, d=PD)
            nc.vector.tensor_reduce(
                vcpart[:, b:b + 1], sr[:, 1:1 + HS, 1:1 + W, 1:1 + D],
                axis=mybir.AxisListType.XYZ, op=AO.add)

    vc_bin = dram.tile([128, 6], F32)
    vc_bout = dram.tile([128, 6], F32)
    nc.gpsimd.dma_start(vc_bin[:], vcpart[:])
    cc("AllReduce", AO.add, vc_bin, vc_bout)
    nc.gpsimd.dma_start(vcpart[:], vc_bout[:])
    nc.vector.tensor_scalar_mul(combined[:, 0:6], vcpart[:],
                                1.0 / (H * W * D))

    if _STOP == "A":
        small_cm.__exit__(None, None, None)
        dram_cm.__exit__(None, None, None)
        return
    # ---------------- Phase B: context GEMVs ------------------------------
    def gemv(dst, dst_cols, wT_dram, kin, x_sb, x_cols, psum_pool, wpool,
             evict):
        nk = kin // 128
        nm = len(dst_cols)
        psums = [psum_pool.tile([128, 1], F32, tag=f"gvp{m}", name=f"gvp{m}")
                 for m in range(nm)]
        for k in range(nk):
            wt = wpool.tile([128, 128 * nm], F32, tag="gemvw", name="gemvw")
            nc.sync.dma_start(wt[:], wT_dram[128 * k:128 * (k + 1), :])
            for mi in range(nm):
                nc.tensor.matmul(
                    psums[mi][:], wt[:, 128 * mi:128 * (mi + 1)],
                    x_sb[:, x_cols[k]:x_cols[k] + 1],
                    start=(k == 0), stop=(k == nk - 1),
                    skip_group_check=True)
        for mi in range(nm):
            evict(dst, dst_cols[mi], mi, psums[mi])

    with tc.tile_pool(name="gemvw", bufs=3) as wpool, \
         tc.tile_pool(name="gemvp", bufs=1, space="PSUM") as gpsum:

        def ev_add(bias_sb):
            def f(dst, col, mi, ps):
                nc.vector.tensor_scalar(
                    dst[:, col:col + 1], ps[:], bias_sb[:, mi:mi + 1], None,
                    op0=AO.add)
            return f

        gemv(combined, [6, 7, 8, 9, 10, 11], io["aT_d"], C, text_sb,
             list(range(6)), gpsum, wpool, ev_add(bcomp_sb))

        def ev_sig(dst, col, mi, ps):
            nc.scalar.activation(dst[:, col:col + 1], ps[:], ACTF.Sigmoid,
                                 bias=modb_sb[:, mi:mi + 1], scale=1.0)
        gemv(mod_sb, list(range(6)), io["modT_d"], 2 * C, combined,
             list(range(12)), gpsum, wpool, ev_sig)

        def ev_relu(dst, col, mi, ps):
            nc.scalar.activation(dst[:, col:col + 1], ps[:], ACTF.Relu,
                                 bias=b1_sb[:, mi:mi + 1], scale=1.0)
        gemv(h1sh, [0, 1, 2], io["w1T_d"], 2 * C, combined, list(range(12)),
             gpsum, wpool, ev_relu)

        h1_bin = dram.tile([1, H1_SH], F32)
        h1_bout = dram.tile([NCORES, H1_SH], F32)
        nc.gpsimd.dma_start(
            h1_bin.rearrange("a (m p) -> (a p) m", p=128), h1sh[:])
        cc("AllGather", AO.bypass, h1_bin, h1_bout)
        h1v = h1_bout.rearrange("r s -> (r s)").rearrange(
            "(k p) -> p k", p=128)
        if W2_BF16:
            nc.gpsimd.dma_start(h1full[:], h1v)  # casting DMA (gpsimd)
        else:
            nc.gpsimd.dma_start(h1full[:], h1v.bitcast(F32R))

    # kp shard = kn_w2_shard @ h1 + b2 (fp32r, streamed from HBM)
    NKP = 6
    KPW = KP_SH // NKP  # 432
    NK2 = 4 * C // 128  # 24
    with tc.tile_pool(name="w2s", bufs=12) as w2pool, \
         tc.tile_pool(name="kpps", bufs=1, space="PSUM") as kpps:
        kpp = [kpps.tile([1, KPW], F32, tag=f"kpp{n}", name=f"kpp{n}") for n in range(NKP)]
        for k in range(NK2 if "W2" not in _SKIP else 1):
            wt = w2pool.tile([128, KP_SH], W2DT if W2_BF16 else F32R,
                             tag="w2t", name="w2t")
            w2src = io["w2T_d"][128 * k:128 * (k + 1), :]
            nc.sync.dma_start(wt[:], w2src if W2_BF16 else w2src.bitcast(F32R))
            for n in range(NKP):
                nc.tensor.matmul(
                    kpp[n][:], h1full[:, k:k + 1],
                    wt[:, KPW * n:KPW * (n + 1)],
                    start=(k == 0),
                    stop=(k == (NK2 if "W2" not in _SKIP else 1) - 1),
                    skip_group_check=True)
        for n in range(NKP):
            nc.vector.tensor_tensor(
                kp_sb[:, KPW * n:KPW * (n + 1)], kpp[n][:],
                b2_sb[:, KPW * n:KPW * (n + 1)], AO.add)

    kp_bin = dram.tile([1, KP_SH], F32)
    kp_bout = dram.tile([NCORES, KP_SH], F32)
    nc.gpsimd.dma_start(kp_bin[:], kp_sb[:])
    cc("AllGather", AO.bypass, kp_bin, kp_bout)
    kp_flat = kp_bout.rearrange("r s -> (r s)")
    for b in range(NB):
        nc.sync.dma_start(
            keff[:, 27 * b:27 * (b + 1)],
            kp_flat[128 * 27 * b:128 * 27 * (b + 1)]
            .rearrange("(p t) -> p t", t=27))
    for b in range(NB):
        nc.vector.tensor_scalar(
            keff[:, 27 * b:27 * (b + 1)], keff[:, 27 * b:27 * (b + 1)],
            mod_sb[:, b:b + 1], None, op0=AO.mult)

    if _STOP == "B":
        small_cm.__exit__(None, None, None)
        dram_cm.__exit__(None, None, None)
        return
    # ---------------- Phase C: depthwise 3x3x3 conv -----------------------
    xc_cm = tc.tile_pool(name="xc", bufs=1)
    xc_pool = xc_cm.__enter__()
    xcs = [xc_pool.tile([128, NVOX], CDT, tag=f"xc{b}", name=f"xc{b}")
           for b in range(NB)]
    HVOX = NVOX // 2
    with tc.tile_pool(name="slabC", bufs=2) as slabC_pool, \
         tc.tile_pool(name="dveacc", bufs=1) as acc_pool, \
         tc.tile_pool(name="sqscr", bufs=1) as sq_pool, \
         tc.tile_pool(name="diag", bufs=1) as diag_pool, \
         tc.tile_pool(name="convp", bufs=4, space="PSUM") as conv_psum:
        # build every diagonal tile up front so the ACT queue never blocks
        # the next block's PE taps behind a DVE-gated sumsq
        pe_taps = PE_TAPS if "PE" not in _SKIP else PE_TAPS[:1]
        diags = {}
        for b in range(NB):
            kb = keff[:, 27 * b:27 * (b + 1)]
            for t in pe_taps:
                dg = diag_pool.tile([128, 128], CDT, tag=f"diag{b}_{t}",
                                    name=f"diag{b}_{t}")
                nc.scalar.activation(dg[:], eye_sb[:], ACTF.Copy,
                                     bias=0.0, scale=kb[:, t:t + 1])
                diags[(b, t)] = dg

        for b in range(NB):
            st = slabC_pool.tile([128, SLABF], CDT, tag="slabC", name="slabC")
            ssrc = slab_d[128 * b:128 * (b + 1), :]
            nc.sync.dma_start(st[:], ssrc if CONV_BF16 else ssrc.bitcast(F32R))
            sr = st.rearrange("p (h w d) -> p h w d", h=PH, w=PW, d=PD)
            if CONV_BF16:
                srf = sr
            else:
                srf = st.bitcast(F32).rearrange("p (h w d) -> p h w d",
                                                h=PH, w=PW, d=PD)
            kb = keff[:, 27 * b:27 * (b + 1)]

            acc = acc_pool.tile([128, NVOX], F32, tag="acc", name="acc")
            accr = acc.rearrange("p (h w d) -> p h w d", h=HS, w=W, d=D)
            for hp in range(HS):
                for i, t in enumerate(
                        DVE_TAPS if "DVE" not in _SKIP else DVE_TAPS[:1]):
                    a, bb_, c3 = t // 9, (t // 3) % 3, t % 3
                    tv = srf[:, a + hp, bb_:bb_ + W, c3:c3 + D]
                    av = accr[:, hp]
                    if i == 0:
                        nc.vector.tensor_scalar(
                            av, tv, kb[:, t:t + 1], None, op0=AO.mult)
                    else:
                        nc.vector.scalar_tensor_tensor(
                            out=av, in0=tv, scalar=kb[:, t:t + 1],
                            in1=av, op0=AO.mult, op1=AO.add)

            xc = xcs[b]
            PVOX = NVOX // HS  # 1024 voxels per h-plane
            for hp4 in range(HS):
                ps = conv_psum.tile([128, PVOX], F32, tag="convp",
                                    name="convp")
                psr = ps.rearrange("p (w d) -> p w d", w=W, d=D)
                for ci, t in enumerate(pe_taps):
                    tv = _tap_view(sr, t)
                    first, last = ci == 0, ci == len(pe_taps) - 1
                    for wh in range(2):
                        nc.tensor.matmul(
                            psr[:, 16 * wh:16 * (wh + 1), :],
                            diags[(b, t)],
                            tv[:, hp4:hp4 + 1, 16 * wh:16 * (wh + 1), :],
                            start=first, stop=last,
                            skip_group_check=True)
                nc.vector.scalar_tensor_tensor(
                    out=xc[:, PVOX * hp4:PVOX * (hp4 + 1)],
                    in0=ps[:], scalar=1.0,
                    in1=acc[:, PVOX * hp4:PVOX * (hp4 + 1)],
                    op0=AO.mult, op1=AO.add,
                    accum_out=chsum[:, 4 * b + hp4:4 * b + hp4 + 1])
                xcv = xc if CONV_BF16 else xc.bitcast(F32)
                sqs = sq_pool.tile([128, PVOX], BF16, tag="sqs", name="sqs")
                nc.scalar.activation(
                    sqs[:], xcv[:, PVOX * hp4:PVOX * (hp4 + 1)],
                    ACTF.Square,
                    accum_out=chsq[:, 4 * b + hp4:4 * b + hp4 + 1])

    if _STOP == "C":
        xc_cm.__exit__(None, None, None)
        small_cm.__exit__(None, None, None)
        dram_cm.__exit__(None, None, None)
        return
    # ---------------- Phase D: GroupNorm stats + affine fold --------------
    with tc.tile_pool(name="statp", bufs=1, space="PSUM") as stat_psum:
        gps = stat_psum.tile([G, 2], F32, tag="gstat", name="gstat")
        for b in range(NB):
            nc.vector.tensor_reduce(
                chstats[:, 0:1], chsum[:, 4 * b:4 * b + 4],
                axis=mybir.AxisListType.X, op=AO.add)
            nc.vector.tensor_reduce(
                chstats[:, 1:2], chsq[:, 4 * b:4 * b + 4],
                axis=mybir.AxisListType.X, op=AO.add)
            nc.tensor.matmul(gps[:], ind_sb[:, G * b:G * (b + 1)],
                             chstats[:], start=(b == 0), stop=(b == NB - 1),
                             skip_group_check=True)
        nc.vector.tensor_copy(gstat[:], gps[:])

        gn_bin = dram.tile([G, 2], F32)
        gn_bout = dram.tile([G, 2], F32)
        nc.gpsimd.dma_start(gn_bin[:], gstat[:])
        cc("AllReduce", AO.add, gn_bin, gn_bout)
        nc.gpsimd.dma_start(gstat[:], gn_bout[:])

        # gv[:,0] = 1/sqrt(var+eps), gv[:,1] = -mu
        nc.vector.tensor_scalar_mul(gv_sb[:, 1:2], gstat[:, 0:1],
                                    -1.0 / NG_TOT)
        nc.vector.tensor_scalar_mul(gv_sb[:, 2:3], gstat[:, 1:2],
                                    1.0 / NG_TOT)
        nc.vector.scalar_tensor_tensor(
            out=gv_sb[:, 3:4], in0=gv_sb[:, 1:2], scalar=gv_sb[:, 1:2],
            in1=gv_sb[:, 2:3], op0=AO.mult, op1=AO.subtract)
        nc.vector.tensor_scalar(gv_sb[:, 3:4], gv_sb[:, 3:4], -1.0, EPS,
                                op0=AO.mult, op1=AO.add)
        nc.scalar.activation(gv_sb[:, 3:4], gv_sb[:, 3:4], ACTF.Sqrt)
        nc.vector.reciprocal(gv_sb[:, 0:1], gv_sb[:, 3:4])

        for b in range(NB):
            bps = stat_psum.tile([128, 2], F32, tag="bcast", name="bcast")
            nc.tensor.matmul(bps[:], sel_sb[:, 128 * b:128 * (b + 1)],
                             gv_sb[:, 0:2], start=True, stop=True)
            nc.vector.tensor_tensor(s_sb[:, b:b + 1], gnw_sb[:, b:b + 1],
                                    bps[:, 0:1], AO.mult)
            nc.vector.scalar_tensor_tensor(
                out=t_sb[:, b:b + 1], in0=s_sb[:, b:b + 1],
                scalar=bps[:, 1:2], in1=gnb_sb[:, b:b + 1],
                op0=AO.mult, op1=AO.add)


    # ---------------- Phase E: bias GEMV + final 1x1x1 GEMM ---------------
    with tc.tile_pool(name="wts", bufs=1) as wts_pool, \
         tc.tile_pool(name="ysb", bufs=4) as y_pool, \
         tc.tile_pool(name="bpp_ps", bufs=1, space="PSUM") as bpp_psum, \
         tc.tile_pool(name="gemmp", bufs=3, space="PSUM") as gemm_psum:
        bps2 = bpp_psum.tile([128, 6], F32, tag="bppp", name="bppp")
        wkt = []
        for kb2 in range(NB):
            wt = wts_pool.tile([128, C], CDT, tag=f"wts{kb2}",
                               name=f"wts{kb2}")
            wsrc = io["convT_d"][128 * kb2:128 * (kb2 + 1), :]
            nc.sync.dma_start(wt[:], wsrc if CONV_BF16 else wsrc.bitcast(F32R))
            wkt.append(wt)
            wtb = wt if CONV_BF16 else wt.bitcast(F32)
            ttb = t_sb if CONV_BF16 else t_sb.bitcast(F32)
            for mb in range(NB):
                nc.tensor.matmul(
                    bps2[:, mb:mb + 1], wtb[:, 128 * mb:128 * (mb + 1)],
                    ttb[:, kb2:kb2 + 1],
                    start=(kb2 == 0), stop=(kb2 == NB - 1),
                    skip_group_check=True)
        nc.vector.tensor_tensor(bpp_sb[:], bps2[:], convb_sb[:], AO.add)

        # scale W columns (contraction rows) by the GroupNorm s factor;
        # must happen after the b'' GEMV, which uses the unscaled weights
        for kb2 in range(NB):
            nc.vector.tensor_scalar(
                wkt[kb2][:], wkt[kb2][:], s_sb[:, kb2:kb2 + 1], None,
                op0=AO.mult)

        NCH = 8
        CW = NVOX // NCH  # 512
        for mb in range(NB if "GEMM" not in _SKIP else 1):
            for nch in range(NCH):
                ps = gemm_psum.tile([128, CW], F32, tag="gemmp", name="gemmp")
                for kb2 in range(NB):
                    nc.tensor.matmul(
                        ps[:], wkt[kb2][:, 128 * mb:128 * (mb + 1)],
                        xcs[kb2][:, CW * nch:CW * (nch + 1)],
                        start=(kb2 == 0), stop=(kb2 == NB - 1))
                ysb = y_pool.tile([128, CW], F32, tag="ysb", name="ysb")
                nc.vector.tensor_scalar(
                    ysb[:], ps[:], bpp_sb[:, mb:mb + 1], None, op0=AO.add)
                nc.sync.dma_start(
                    out_d[128 * mb:128 * (mb + 1), CW * nch:CW * (nch + 1)],
                    ysb[:])

    xc_cm.__exit__(None, None, None)
    dram_cm.__exit__(None, None, None)
    small_cm.__exit__(None, None, None)


def _host_prep(inputs):
    import ml_dtypes
    bf = ml_dtypes.bfloat16
    cdt = bf if CONV_BF16 else np.float32
    w2dt = bf if W2_BF16 else np.float32
    f = np.float32
    vf = np.ascontiguousarray(np.asarray(inputs["visual_feat"])[0].astype(f))
    text = np.asarray(inputs["text_feat"][0]).astype(np.float64)

    tpw = np.asarray(inputs["text_proj_w"]).astype(np.float64)
    tpb = np.asarray(inputs["text_proj_b"]).astype(np.float64)
    ipw = np.asarray(inputs["in_proj_w"]).astype(np.float64)
    ipb = np.asarray(inputs["in_proj_b"]).astype(np.float64)
    opw = np.asarray(inputs["out_proj_w"]).astype(np.float64)
    opb = np.asarray(inputs["out_proj_b"]).astype(np.float64)

    wv = ipw[2 * C:3 * C]
    bv = ipb[2 * C:3 * C]
    A = opw @ wv @ tpw
    bcomp = opw @ (wv @ tpb + bv) + opb

    def chunks128(v):
        return np.ascontiguousarray(
            np.asarray(v, np.float64).reshape(6, 128).T.astype(f))

    ind = np.zeros((C, G), f)
    for c in range(C):
        ind[c, c // GD] = 1.0

    common = {
        "aT": np.ascontiguousarray(A.T.astype(f)),
        "textv": chunks128(text),
        "bcomp": chunks128(bcomp),
        "modT": np.ascontiguousarray(np.asarray(inputs["mod_w"]).T.astype(f)),
        "modb": chunks128(inputs["mod_b"]),
        "convT": np.ascontiguousarray(
            np.asarray(inputs["conv_w"]).reshape(C, C).T.astype(cdt)),
        "convb": chunks128(inputs["conv_b"]),
        "gnw": np.asarray(inputs["gn_w"], f).copy(),
        "gnb": np.asarray(inputs["gn_b"], f).copy(),
        "eye": np.eye(128, dtype=f),
        "ind": np.ascontiguousarray(
            ind.reshape(NB, 128, G).transpose(1, 0, 2).reshape(128, NB * G)),
        "sel": np.ascontiguousarray(ind.T),
    }

    vf_pad = np.pad(vf, ((0, 0), (1, 1), (1, 1), (1, 1)))
    w1 = np.asarray(inputs["kn_w1"], f)
    b1 = np.asarray(inputs["kn_b1"], f)
    w2 = np.asarray(inputs["kn_w2"], f)
    b2 = np.asarray(inputs["kn_b2"], f)

    in_maps = []
    for j in range(NCORES):
        m = dict(common)
        m["slab"] = np.ascontiguousarray(
            vf_pad[:, 4 * j:4 * j + PH, :, :].reshape(C, SLABF).astype(cdt))
        m["w1T"] = np.ascontiguousarray(w1[H1_SH * j:H1_SH * (j + 1)].T)
        m["b1"] = np.ascontiguousarray(
            b1[H1_SH * j:H1_SH * (j + 1)].reshape(3, 128).T)
        m["w2T"] = np.ascontiguousarray(
            w2[KP_SH * j:KP_SH * (j + 1)].T.astype(w2dt))
        m["b2"] = np.ascontiguousarray(
            b2[KP_SH * j:KP_SH * (j + 1)].reshape(1, KP_SH))
        in_maps.append(m)
    return in_maps


def kernel(**inputs):
    if "nc" not in _BUILD_CACHE:
        _BUILD_CACHE["nc"] = build_program(with_collectives=True)
    nc = _BUILD_CACHE["nc"]
    in_maps = _host_prep(inputs)
    res = bass_utils.run_bass_kernel_spmd(
        nc, in_maps, core_ids=list(range(NCORES)))
    out = np.empty((1, C, H, W, D), np.float32)
    for j in range(NCORES):
        out[0, :, 4 * j:4 * j + 4, :, :] = \
            res.results[j]["out"].reshape(C, HS, W, D)
    return out



# revision 6
# speedup vs baseline: 1.4927x; 1.4927x over previous
"""CrossModalAdaptiveFusion Trainium2 kernel (8 NeuronCores, SPMD).

Sharding: the 32^3 volume is split into 8 H-slabs of 4 planes; each core
receives its 4 planes plus the 2 halo planes unpadded (bf16) and builds the
zero-padded slab in SBUF, so the depthwise conv, GroupNorm reduction and the
final 1x1x1 projection all stay core-local.

The tiny context path (avg-pool -> attention -> kernel-MLP -> modulation,
~0.13 GFLOP, 3% of total work) is folded on the host into the 768x27
effective depthwise kernels `keff = kp * sigmoid(mod)`, so the 63M-param
kn_w2 never crosses the host->device link. The device runs the heavy 97%:
the depthwise 3x3x3 conv (split between the PE via diagonal-matmul
accumulation in PSUM and the DVE via a scalar_tensor_tensor FMA chain),
GroupNorm folded into a per-channel affine, and the 768x768 x 4096-voxel
output GEMM. Cross-core traffic is two tiny collectives: an AllGather of the
row-sharded conv_w.T (each core uploads 1/8) and an AllReduce of the 12x2
GroupNorm stats. Output is written bf16 to halve the device->host link cost.
"""
import sys

sys.path.insert(0, "/opt/trn_rl_repo")

import numpy as np

import concourse.bass as bass
import concourse.mybir as mybir
from concourse import tile
from concourse import bass_utils

F32 = mybir.dt.float32
BF16 = mybir.dt.bfloat16
AO = mybir.AluOpType
ACTF = mybir.ActivationFunctionType

C = 768
G = 12
GD = C // G          # 64 channels per group
H = W = D = 32
NCORES = 8
HS = H // NCORES     # 4 H-planes per core
NB = C // 128        # 6 channel blocks
PH, PW, PD = HS + 2, W + 2, D + 2   # padded slab dims: 6 x 34 x 34
SLABF = PH * PW * PD                # 6936 free elements per channel
VOWNF = PH * W * D                  # 6144 unpadded elements shipped
NVOX = HS * W * D                   # 4096 voxels per core
NG_TOT = GD * H * W * D             # element count per GroupNorm group
CSH = C // NCORES                   # 96 conv_w.T rows per core
EPS = 1e-5

# Tap split between engines: DVE runs an FMA chain, the PE runs diagonal
# matmuls accumulating in PSUM.
DVE_TAPS = list(range(7))
PE_TAPS = [t for t in range(27) if t not in DVE_TAPS]

_BUILD_CACHE = {}


def split_multi_waits(nc, max_waits=1):
    """The walrus build in this container accepts at most one sync wait per
    instruction; Tile attaches several. Split the extras into standalone
    single-wait EventSemaphore instructions on the same engine."""
    for bb in nc.main_func.blocks:
        new_list = []
        for inst in bb.instructions:
            si = inst.sync_info
            waits = list(si.on_wait) if si and si.on_wait else []
            if len(waits) > max_waits:
                keep, move = waits[:max_waits], waits[max_waits:]
                for k, w in enumerate(move):
                    ev = mybir.InstEventSemaphore(
                        name=f"{inst.name}-ws{k}", ins=[], outs=[])
                    ev.engine = inst.engine
                    ev.sync_info = mybir.SyncInfo(on_wait=[w], on_update=[])
                    new_list.append(ev)
                si.on_wait = keep
            new_list.append(inst)
        bb.instructions[:] = new_list


def _tap_view(slab_r, t):
    """Shifted [128, 4, 32, 32] view of the padded slab for tap t."""
    a, b, c3 = t // 9, (t // 3) % 3, t % 3
    return slab_r[:, a:a + HS, b:b + W, c3:c3 + D]


def build_program(with_collectives=True):
    nc = bass.Bass("TRN2", target_bir_lowering=False, debug=False,
                   num_devices=NCORES)

    def din(name, shape, dt=F32):
        return nc.dram_tensor(name, shape, dt, kind="ExternalInput").ap()

    io = {}
    io["vown_d"] = din("vown", [C, SLABF], BF16)  # padded 6-plane H-slab
    io["keff_d"] = din("keff", [128, 27 * NB])    # host-folded kp * mod
    io["convT_d"] = din("convT", [CSH, C], BF16)  # conv_w.T row shard
    io["convb_d"] = din("convb", [128, NB])       # conv_b chunks
    io["gnw_d"] = din("gnw", [C])                 # gn_w
    io["gnb_d"] = din("gnb", [C])                 # gn_b
    io["eye_d"] = din("eye", [128, 128])          # identity, for diag builds
    io["ind_d"] = din("ind", [128, G * NB])       # channel->group indicator
    io["sel_d"] = din("sel", [G, C])              # group->channel selector
    io["out_d"] = nc.dram_tensor("out", [C, NVOX], BF16,
                                 kind="ExternalOutput").ap()

    with tile.TileContext(nc) as tc:
        _emit(nc, tc, io, with_collectives)

    split_multi_waits(nc)
    return nc


def _emit(nc, tc, io, with_collectives):
    out_d = io["out_d"]
    RG = [list(range(NCORES))]

    def cc(kind, op, in_ap, out_ap):
        if with_collectives:
            nc.gpsimd.collective_compute(
                kind, op, replica_groups=RG,
                ins=[in_ap.opt()], outs=[out_ap.opt()])
        else:
            shp = in_ap.shape
            nc.gpsimd.dma_start(
                out_ap[tuple(slice(0, s) for s in shp)], in_ap[:])

    small_cm = tc.tile_pool(name="small", bufs=1)
    small = small_cm.__enter__()

    keff = small.tile([128, 27 * NB], F32, tag="keff", name="keff")
    chsum = small.tile([128, 24], F32, tag="chsum", name="chsum")
    chsq = small.tile([128, 24], F32, tag="chsq", name="chsq")
    eye_sb = small.tile([128, 128], F32, tag="eye", name="eye")
    gnw_sb = small.tile([128, NB], F32, tag="gnw", name="gnw")
    gnb_sb = small.tile([128, NB], F32, tag="gnb", name="gnb")
    convb_sb = small.tile([128, NB], F32, tag="convb", name="convb")
    ind_sb = small.tile([128, G * NB], F32, tag="ind", name="ind")
    sel_sb = small.tile([G, 128 * NB], F32, tag="sel", name="sel")
    s_sb = small.tile([128, NB], F32, tag="s", name="s")
    t_sb = small.tile([128, NB], BF16, tag="t", name="t")
    gv_sb = small.tile([G, 4], F32, tag="gv", name="gv")
    bpp_sb = small.tile([128, NB], F32, tag="bpp", name="bpp")
    chstats = small.tile([128, 2], F32, tag="chstats", name="chstats")
    gstat = small.tile([G, 2], F32, tag="gstat_sb", name="gstat_sb")

    dram_cm = tc.tile_pool(name="dram", bufs=1, space="DRAM")
    dram = dram_cm.__enter__()

    # Launch the conv_w.T AllGather first: it only needs the input DRAM
    # tensor, so it overlaps with the whole conv phase.
    convT_stage = dram.tile([CSH, C], BF16)
    convT_full = dram.tile([C, C], BF16)
    nc.gpsimd.dma_start(convT_stage[:], io["convT_d"][:])
    cc("AllGather", AO.bypass, convT_stage, convT_full)

    nc.sync.dma_start(keff[:], io["keff_d"][:])
    nc.sync.dma_start(eye_sb[:], io["eye_d"][:])
    nc.sync.dma_start(gnw_sb[:], io["gnw_d"].rearrange("(k p) -> p k", p=128))
    nc.sync.dma_start(gnb_sb[:], io["gnb_d"].rearrange("(k p) -> p k", p=128))
    nc.sync.dma_start(convb_sb[:], io["convb_d"][:])
    nc.sync.dma_start(ind_sb[:], io["ind_d"][:])
    nc.sync.dma_start(sel_sb[:], io["sel_d"][:])

    # ---------------- Phase C: depthwise 3x3x3 conv -----------------------
    xc_cm = tc.tile_pool(name="xc", bufs=1)
    xc_pool = xc_cm.__enter__()
    xcs = [xc_pool.tile([128, NVOX], BF16, tag=f"xc{b}", name=f"xc{b}")
           for b in range(NB)]
    with tc.tile_pool(name="slabC", bufs=2) as slabC_pool, \
         tc.tile_pool(name="dveacc", bufs=1) as acc_pool, \
         tc.tile_pool(name="sqscr", bufs=1) as sq_pool, \
         tc.tile_pool(name="diag", bufs=1) as diag_pool, \
         tc.tile_pool(name="convp", bufs=4, space="PSUM") as conv_psum:
        # build every diagonal tile up front so the ACT queue never blocks
        # the next block's PE taps behind a DVE-gated sumsq
        diags = {}
        for b in range(NB):
            kb = keff[:, 27 * b:27 * (b + 1)]
            for t in PE_TAPS:
                dg = diag_pool.tile([128, 128], BF16, tag=f"diag{b}_{t}",
                                    name=f"diag{b}_{t}")
                nc.scalar.activation(dg[:], eye_sb[:], ACTF.Copy,
                                     bias=0.0, scale=kb[:, t:t + 1])
                diags[(b, t)] = dg

        for b in range(NB):
            st = slabC_pool.tile([128, SLABF], BF16, tag="slabC", name="slabC")
            sr = st.rearrange("p (h w d) -> p h w d", h=PH, w=PW, d=PD)
            nc.sync.dma_start(st[:], io["vown_d"][128 * b:128 * (b + 1), :])
            kb = keff[:, 27 * b:27 * (b + 1)]

            acc = acc_pool.tile([128, NVOX], F32, tag="acc", name="acc")
            accr = acc.rearrange("p (h w d) -> p h w d", h=HS, w=W, d=D)
            for hp in range(HS):
                for i, t in enumerate(DVE_TAPS):
                    a, bb_, c3 = t // 9, (t // 3) % 3, t % 3
                    tv = sr[:, a + hp, bb_:bb_ + W, c3:c3 + D]
                    av = accr[:, hp]
                    if i == 0:
                        nc.vector.tensor_scalar(
                            av, tv, kb[:, t:t + 1], None, op0=AO.mult)
                    else:
                        nc.vector.scalar_tensor_tensor(
                            out=av, in0=tv, scalar=kb[:, t:t + 1],
                            in1=av, op0=AO.mult, op1=AO.add)

            xc = xcs[b]
            PVOX = NVOX // HS  # 1024 voxels per h-plane
            for hp4 in range(HS):
                ps = conv_psum.tile([128, PVOX], F32, tag="convp",
                                    name="convp")
                psr = ps.rearrange("p (w d) -> p w d", w=W, d=D)
                for ci, t in enumerate(PE_TAPS):
                    tv = _tap_view(sr, t)
                    first, last = ci == 0, ci == len(PE_TAPS) - 1
                    for wh in range(2):
                        nc.tensor.matmul(
                            psr[:, 16 * wh:16 * (wh + 1), :],
                            diags[(b, t)],
                            tv[:, hp4:hp4 + 1, 16 * wh:16 * (wh + 1), :],
                            start=first, stop=last,
                            skip_group_check=True)
                nc.vector.scalar_tensor_tensor(
                    out=xc[:, PVOX * hp4:PVOX * (hp4 + 1)],
                    in0=ps[:], scalar=1.0,
                    in1=acc[:, PVOX * hp4:PVOX * (hp4 + 1)],
                    op0=AO.mult, op1=AO.add,
                    accum_out=chsum[:, 4 * b + hp4:4 * b + hp4 + 1])
                sqs = sq_pool.tile([128, PVOX], BF16, tag="sqs", name="sqs")
                nc.scalar.activation(
                    sqs[:], xc[:, PVOX * hp4:PVOX * (hp4 + 1)],
                    ACTF.Square,
                    accum_out=chsq[:, 4 * b + hp4:4 * b + hp4 + 1])

    # ---------------- Phase D: GroupNorm stats + affine fold --------------
    with tc.tile_pool(name="statp", bufs=1, space="PSUM") as stat_psum:
        gps = stat_psum.tile([G, 2], F32, tag="gstat", name="gstat")
        for b in range(NB):
            nc.vector.tensor_reduce(
                chstats[:, 0:1], chsum[:, 4 * b:4 * b + 4],
                axis=mybir.AxisListType.X, op=AO.add)
            nc.vector.tensor_reduce(
                chstats[:, 1:2], chsq[:, 4 * b:4 * b + 4],
                axis=mybir.AxisListType.X, op=AO.add)
            nc.tensor.matmul(gps[:], ind_sb[:, G * b:G * (b + 1)],
                             chstats[:], start=(b == 0), stop=(b == NB - 1),
                             skip_group_check=True)
        nc.vector.tensor_copy(gstat[:], gps[:])

        gn_bin = dram.tile([G, 2], F32)
        gn_bout = dram.tile([G, 2], F32)
        nc.gpsimd.dma_start(gn_bin[:], gstat[:])
        cc("AllReduce", AO.add, gn_bin, gn_bout)
        nc.gpsimd.dma_start(gstat[:], gn_bout[:])

        # gv[:,0] = 1/sqrt(var+eps), gv[:,1] = -mu
        nc.vector.tensor_scalar_mul(gv_sb[:, 1:2], gstat[:, 0:1],
                                    -1.0 / NG_TOT)
        nc.vector.tensor_scalar_mul(gv_sb[:, 2:3], gstat[:, 1:2],
                                    1.0 / NG_TOT)
        nc.vector.scalar_tensor_tensor(
            out=gv_sb[:, 3:4], in0=gv_sb[:, 1:2], scalar=gv_sb[:, 1:2],
            in1=gv_sb[:, 2:3], op0=AO.mult, op1=AO.subtract)
        nc.vector.tensor_scalar(gv_sb[:, 3:4], gv_sb[:, 3:4], -1.0, EPS,
                                op0=AO.mult, op1=AO.add)
        nc.scalar.activation(gv_sb[:, 3:4], gv_sb[:, 3:4], ACTF.Sqrt)
        nc.vector.reciprocal(gv_sb[:, 0:1], gv_sb[:, 3:4])

        for b in range(NB):
            bps = stat_psum.tile([128, 2], F32, tag="bcast", name="bcast")
            nc.tensor.matmul(bps[:], sel_sb[:, 128 * b:128 * (b + 1)],
                             gv_sb[:, 0:2], start=True, stop=True)
            nc.vector.tensor_tensor(s_sb[:, b:b + 1], gnw_sb[:, b:b + 1],
                                    bps[:, 0:1], AO.mult)
            nc.vector.scalar_tensor_tensor(
                out=t_sb[:, b:b + 1], in0=s_sb[:, b:b + 1],
                scalar=bps[:, 1:2], in1=gnb_sb[:, b:b + 1],
                op0=AO.mult, op1=AO.add)

    # ---------------- Phase E: bias GEMV + final 1x1x1 GEMM ---------------
    with tc.tile_pool(name="wts", bufs=1) as wts_pool, \
         tc.tile_pool(name="ysb", bufs=4) as y_pool, \
         tc.tile_pool(name="bpp_ps", bufs=1, space="PSUM") as bpp_psum, \
         tc.tile_pool(name="gemmp", bufs=3, space="PSUM") as gemm_psum:
        bps2 = bpp_psum.tile([128, NB], F32, tag="bppp", name="bppp")
        wkt = []
        for kb2 in range(NB):
            wt = wts_pool.tile([128, C], BF16, tag=f"wts{kb2}",
                               name=f"wts{kb2}")
            nc.sync.dma_start(wt[:], convT_full[128 * kb2:128 * (kb2 + 1), :])
            wkt.append(wt)
            for mb in range(NB):
                nc.tensor.matmul(
                    bps2[:, mb:mb + 1], wt[:, 128 * mb:128 * (mb + 1)],
                    t_sb[:, kb2:kb2 + 1],
                    start=(kb2 == 0), stop=(kb2 == NB - 1),
                    skip_group_check=True)
        nc.vector.tensor_tensor(bpp_sb[:], bps2[:], convb_sb[:], AO.add)

        # scale W columns (contraction rows) by the GroupNorm s factor;
        # must happen after the b'' GEMV, which uses the unscaled weights
        for kb2 in range(NB):
            nc.vector.tensor_scalar(
                wkt[kb2][:], wkt[kb2][:], s_sb[:, kb2:kb2 + 1], None,
                op0=AO.mult)

        NCH = 8
        CW = NVOX // NCH  # 512
        for mb in range(NB):
            for nch in range(NCH):
                ps = gemm_psum.tile([128, CW], F32, tag="gemmp", name="gemmp")
                for kb2 in range(NB):
                    nc.tensor.matmul(
                        ps[:], wkt[kb2][:, 128 * mb:128 * (mb + 1)],
                        xcs[kb2][:, CW * nch:CW * (nch + 1)],
                        start=(kb2 == 0), stop=(kb2 == NB - 1))
                ysb = y_pool.tile([128, CW], BF16, tag="ysb", name="ysb")
                nc.vector.tensor_scalar(
                    ysb[:], ps[:], bpp_sb[:, mb:mb + 1], None, op0=AO.add)
                nc.sync.dma_start(
                    out_d[128 * mb:128 * (mb + 1), CW * nch:CW * (nch + 1)],
                    ysb[:])

    xc_cm.__exit__(None, None, None)
    dram_cm.__exit__(None, None, None)
    small_cm.__exit__(None, None, None)


def _host_context(inputs):
    """The tiny context path, in float64 except the one 63M-MAC matvec."""
    d = np.float64
    f = np.float32
    vf = np.asarray(inputs["visual_feat"])[0]                  # [C, 32,32,32]
    vc = vf.reshape(C, -1).mean(axis=1, dtype=d)               # [C]
    text = np.asarray(inputs["text_feat"][0]).astype(d)

    tpw = np.asarray(inputs["text_proj_w"]).astype(d)
    tpb = np.asarray(inputs["text_proj_b"]).astype(d)
    ipw = np.asarray(inputs["in_proj_w"]).astype(d)
    ipb = np.asarray(inputs["in_proj_b"]).astype(d)
    opw = np.asarray(inputs["out_proj_w"]).astype(d)
    opb = np.asarray(inputs["out_proj_b"]).astype(d)

    tp = tpw @ text + tpb
    # softmax over a single key is exactly 1 -> attn == v
    v = ipw[2 * C:] @ tp + ipb[2 * C:]
    attn_context = opw @ v + opb
    combined = np.concatenate([vc, attn_context])              # [2C]

    w1 = np.asarray(inputs["kn_w1"])
    b1 = np.asarray(inputs["kn_b1"]).astype(d)
    h1 = np.maximum(w1 @ combined + b1, 0.0)                   # [4C]
    w2 = np.asarray(inputs["kn_w2"])                           # [KPARAMS, 4C]
    kp = w2 @ h1.astype(f) + np.asarray(inputs["kn_b2"])       # [C*27] f32

    modw = np.asarray(inputs["mod_w"])
    modb = np.asarray(inputs["mod_b"]).astype(d)
    z = modw @ combined + modb
    mod = 1.0 / (1.0 + np.exp(-z))                             # [C]

    keffm = kp.reshape(C, 27).astype(d) * mod[:, None]         # [C, 27]
    return keffm.astype(f)


def _host_prep(inputs):
    import ml_dtypes
    bf = ml_dtypes.bfloat16
    f = np.float32

    keffm = _host_context(inputs)
    keff_in = np.ascontiguousarray(
        keffm.reshape(NB, 128, 27).transpose(1, 0, 2).reshape(128, 27 * NB))

    def chunks128(v):
        return np.ascontiguousarray(
            np.asarray(v, np.float64).reshape(NB, 128).T.astype(f))

    ind = np.zeros((C, G), f)
    for c in range(C):
        ind[c, c // GD] = 1.0

    convT = np.asarray(inputs["conv_w"]).reshape(C, C).T       # [in, out]
    convT_bf = convT.astype(bf)

    common = {
        "keff": keff_in,
        "convb": chunks128(inputs["conv_b"]),
        "gnw": np.asarray(inputs["gn_w"], f).copy(),
        "gnb": np.asarray(inputs["gn_b"], f).copy(),
        "eye": np.eye(128, dtype=f),
        "ind": np.ascontiguousarray(
            ind.reshape(NB, 128, G).transpose(1, 0, 2).reshape(128, NB * G)),
        "sel": np.ascontiguousarray(ind.T),
    }

    # bf16 visual volume zero-padded in H/W/D; each core gets a 6-plane
    # H-window (its 4 planes + halo) of the padded volume
    vf = np.asarray(inputs["visual_feat"])[0]
    vfb = np.zeros((C, H + 2, W + 2, D + 2), bf)
    vfb[:, 1:1 + H, 1:1 + W, 1:1 + D] = vf.astype(bf)

    in_maps = []
    for j in range(NCORES):
        m = dict(common)
        m["vown"] = np.ascontiguousarray(
            vfb[:, HS * j:HS * j + PH]).reshape(C, SLABF)
        m["convT"] = np.ascontiguousarray(convT_bf[CSH * j:CSH * (j + 1)])
        in_maps.append(m)
    return in_maps


def kernel(**inputs):
    if "nc" not in _BUILD_CACHE:
        _BUILD_CACHE["nc"] = build_program(with_collectives=True)
    nc = _BUILD_CACHE["nc"]
    in_maps = _host_prep(inputs)
    res = bass_utils.run_bass_kernel_spmd(
        nc, in_maps, core_ids=list(range(NCORES)))
    out = np.empty((1, C, H, W, D), np.float32)
    for j in range(NCORES):
        out[0, :, HS * j:HS * (j + 1), :, :] = \
            res.results[j]["out"].astype(np.float32).reshape(C, HS, W, D)
    return out
